# revision 16
# baseline (speedup 1.0000x reference)
"""BlockWiseEmbedding gather kernel for 8 Trainium2 NeuronCores.

Strategy: data-parallel over tokens, embedding tables replicated.
out[b, t] = tables_concat[offsets[block_assignment[src[b,t]]] + local_assignment[src[b,t]]]
The host computes the flat row index per token (trivial int math on the
routing tables); each core then performs the memory-bound work: gathering
8192 rows of 2KB from the 200MB concatenated table (indirect DMA, one
descriptor per row) and streaming them to the output, pipelined via Tile.
"""
import functools

import numpy as np

import concourse.bacc as bacc
import concourse.bass as bass
import concourse.mybir as mybir
import concourse.tile as tile
from concourse.bass_utils import run_bass_kernel_spmd

# Problem shape (hardcoded per the harness contract).
BATCH, SEQ = 32, 2048
VOCAB = 100000
DIM = 512
N_CORES = 8
P = 128
TOK_PER_CORE = BATCH * SEQ // N_CORES      # 8192
COLS = TOK_PER_CORE // P                   # 64 tokens per partition
STORE_K = 1                                # gathered columns per output store


@functools.lru_cache(maxsize=1)
def _build():
    nc = bacc.Bacc("TRN2", target_bir_lowering=False, debug=False,
                   dynamic_dma_scratch_size=32768)
    idx_h = nc.dram_tensor("idx", [P, COLS], mybir.dt.int32, kind="ExternalInput")
    tab_h = nc.dram_tensor("table", [VOCAB, DIM], mybir.dt.float32, kind="ExternalInput")
    out_h = nc.dram_tensor(
        "out", [TOK_PER_CORE, DIM], mybir.dt.float32, kind="ExternalOutput"
    )
    # Token t = p*COLS + c lives at SBUF partition p, column c.
    out_v = out_h.ap().rearrange("(p c) d -> p c d", p=P)

    n_batches = COLS // STORE_K
    with tile.TileContext(nc) as tc:
        with (
            tc.tile_pool(name="g", bufs=n_batches) as gpool,
            tc.tile_pool(name="ix", bufs=1) as ixpool,
        ):
            idx_tile = ixpool.tile([P, COLS], mybir.dt.int32)
            nc.sync.dma_start(out=idx_tile[:], in_=idx_h[:])
            # HW indirect DMA moves one 2KB row per partition per
            # instruction; batch STORE_K of them per output store.
            # bufs=n_batches: every batch owns its tile, so the lagging
            # store stream never throttles the gather stream. Stores
            # alternate across the two HWDGE rings (sync/scalar).
            for bi in range(n_batches):
                g = gpool.tile([P, STORE_K * DIM], mybir.dt.float32)
                for j in range(STORE_K):
                    ci = bi * STORE_K + j
                    nc.gpsimd.indirect_dma_start(
                        out=g[:, j * DIM:(j + 1) * DIM],
                        out_offset=None,
                        in_=tab_h[:],
                        in_offset=bass.IndirectOffsetOnAxis(
                            ap=idx_tile[:, ci:ci + 1], axis=0
                        ),
                    )
                store_eng = nc.sync if bi % 2 == 0 else nc.scalar
                store_eng.dma_start(
                    out=out_v[:, bi * STORE_K:(bi + 1) * STORE_K, :], in_=g[:]
                )

    nc.compile()
    return nc


def _prepare(src, block_assignment, local_assignment, tables):
    """Host-side routing: per-token flat row in the concatenated table."""
    src = np.asarray(src).astype(np.int64)
    blk = np.asarray(block_assignment).astype(np.int64)
    loc = np.asarray(local_assignment).astype(np.int64)
    sizes = np.array([t.shape[0] for t in tables], dtype=np.int64)
    offsets = np.concatenate([np.zeros(1, np.int64), np.cumsum(sizes)[:-1]])
    flat = offsets[blk[src]] + loc[src]            # [BATCH, SEQ]
    big = np.ascontiguousarray(
        np.concatenate([np.asarray(t, dtype=np.float32) for t in tables], axis=0)
    )
    return flat.reshape(-1).astype(np.int32), big


def run(inputs, trace=False):
    """Shard, execute on 8 cores, return (full_output, BassKernelResults)."""
    flat, big = _prepare(
        inputs["src"],
        inputs["block_assignment"],
        inputs["local_assignment"],
        [inputs["table0"], inputs["table1"], inputs["table2"], inputs["table3"]],
    )
    in_maps = []
    for c in range(N_CORES):
        idx_c = flat[c * TOK_PER_CORE:(c + 1) * TOK_PER_CORE].reshape(P, COLS)
        in_maps.append({"idx": np.ascontiguousarray(idx_c), "table": big})
    nc = _build()
    # Device execution is occasionally flaky on a fresh NEFF
    # (NRT_EXEC_UNIT_UNRECOVERABLE); an identical retry succeeds.
    last_err = None
    for _ in range(3):
        try:
            res = run_bass_kernel_spmd(
                nc, in_maps, core_ids=list(range(N_CORES)), trace=trace
            )
            break
        except Exception as e:  # noqa: BLE001
            last_err = e
    else:
        raise last_err
    out = np.concatenate([r["out"] for r in res.results], axis=0)
    return out.reshape(BATCH, SEQ, DIM), res


def kernel(**inputs) -> np.ndarray:
    out, _ = run(inputs)
    return out


# revision 17
# speedup vs baseline: 1.0061x; 1.0061x over previous
"""BlockWiseEmbedding gather kernel for 8 Trainium2 NeuronCores.

Strategy: data-parallel over tokens, embedding tables replicated.
out[b, t] = tables_concat[offsets[block_assignment[src[b,t]]] + local_assignment[src[b,t]]]
The host computes the flat row index per token (trivial int math on the
routing tables); each core then performs the memory-bound work: gathering
8192 rows of 2KB from the 200MB concatenated table (indirect DMA, one
descriptor per row) and streaming them to the output, pipelined via Tile.
"""
import functools

import numpy as np

import concourse.bacc as bacc
import concourse.bass as bass
import concourse.mybir as mybir
import concourse.tile as tile
from concourse.bass_utils import run_bass_kernel_spmd

# Problem shape (hardcoded per the harness contract).
BATCH, SEQ = 32, 2048
VOCAB = 100000
DIM = 512
N_CORES = 8
P = 128
TOK_PER_CORE = BATCH * SEQ // N_CORES      # 8192
COLS = TOK_PER_CORE // P                   # 64 tokens per partition
STORE_K = 1                                # gathered columns per output store


@functools.lru_cache(maxsize=1)
def _build():
    nc = bacc.Bacc("TRN2", target_bir_lowering=False, debug=False)
    idx_h = nc.dram_tensor("idx", [P, COLS], mybir.dt.int32, kind="ExternalInput")
    tab_h = nc.dram_tensor("table", [VOCAB, DIM], mybir.dt.float32, kind="ExternalInput")
    out_h = nc.dram_tensor(
        "out", [TOK_PER_CORE, DIM], mybir.dt.float32, kind="ExternalOutput"
    )
    # Token t = p*COLS + c lives at SBUF partition p, column c.
    out_v = out_h.ap().rearrange("(p c) d -> p c d", p=P)

    n_batches = COLS // STORE_K
    with tile.TileContext(nc) as tc:
        with (
            tc.tile_pool(name="g", bufs=n_batches) as gpool,
            tc.tile_pool(name="ix", bufs=1) as ixpool,
        ):
            idx_tile = ixpool.tile([P, COLS], mybir.dt.int32)
            nc.sync.dma_start(out=idx_tile[:], in_=idx_h[:])
            # HW indirect DMA moves one 2KB row per partition per
            # instruction; batch STORE_K of them per output store.
            # bufs=n_batches: every batch owns its tile, so the lagging
            # store stream never throttles the gather stream. Stores
            # alternate across the two HWDGE rings (sync/scalar).
            for bi in range(n_batches):
                g = gpool.tile([P, STORE_K * DIM], mybir.dt.float32)
                for j in range(STORE_K):
                    ci = bi * STORE_K + j
                    nc.gpsimd.indirect_dma_start(
                        out=g[:, j * DIM:(j + 1) * DIM],
                        out_offset=None,
                        in_=tab_h[:],
                        in_offset=bass.IndirectOffsetOnAxis(
                            ap=idx_tile[:, ci:ci + 1], axis=0
                        ),
                    )
                store_eng = nc.sync if bi % 2 == 0 else nc.scalar
                store_eng.dma_start(
                    out=out_v[:, bi * STORE_K:(bi + 1) * STORE_K, :], in_=g[:]
                )

    nc.compile()
    return nc


def _prepare(src, block_assignment, local_assignment, tables):
    """Host-side routing: per-token flat row in the concatenated table."""
    src = np.asarray(src).astype(np.int64)
    blk = np.asarray(block_assignment).astype(np.int64)
    loc = np.asarray(local_assignment).astype(np.int64)
    sizes = np.array([t.shape[0] for t in tables], dtype=np.int64)
    offsets = np.concatenate([np.zeros(1, np.int64), np.cumsum(sizes)[:-1]])
    flat = offsets[blk[src]] + loc[src]            # [BATCH, SEQ]
    big = np.ascontiguousarray(
        np.concatenate([np.asarray(t, dtype=np.float32) for t in tables], axis=0)
    )
    return flat.reshape(-1).astype(np.int32), big


def run(inputs, trace=False):
    """Shard, execute on 8 cores, return (full_output, BassKernelResults)."""
    flat, big = _prepare(
        inputs["src"],
        inputs["block_assignment"],
        inputs["local_assignment"],
        [inputs["table0"], inputs["table1"], inputs["table2"], inputs["table3"]],
    )
    in_maps = []
    for c in range(N_CORES):
        idx_c = flat[c * TOK_PER_CORE:(c + 1) * TOK_PER_CORE].reshape(P, COLS)
        in_maps.append({"idx": np.ascontiguousarray(idx_c), "table": big})
    nc = _build()
    # Device execution is occasionally flaky on a fresh NEFF
    # (NRT_EXEC_UNIT_UNRECOVERABLE); an identical retry succeeds.
    last_err = None
    for _ in range(3):
        try:
            res = run_bass_kernel_spmd(
                nc, in_maps, core_ids=list(range(N_CORES)), trace=trace
            )
            break
        except Exception as e:  # noqa: BLE001
            last_err = e
    else:
        raise last_err
    out = np.concatenate([r["out"] for r in res.results], axis=0)
    return out.reshape(BATCH, SEQ, DIM), res


def kernel(**inputs) -> np.ndarray:
    out, _ = run(inputs)
    return out
